# revision 29
# baseline (speedup 1.0000x reference)
"""Trainium2 Bass kernel for nn_DiversityUncertainty (retrieval_knn).

out = lambda * norm01(entropy(pred)) + norm01(min_l2_dist(U_z, L_z))

Sharding: U_z / pred row-sharded across 8 cores (2048 rows each), L_z
replicated.  Per core (n-orientation: L rows on psum partitions, U rows
on the free dim):

  distance:
    - fp16 GEMM  psum[n, m] = +2 * (L @ U^T) chunk   (PE, fp32 accum)
    - evacuation split across two engines per chunk type:
        ACT chunks: ev = relu(psum + (B - |l|^2))  then DVE
                    acc = max(acc, ev)             (fp16, 2x mode)
        STT chunks: acc = max(acc, psum + (B - |l|^2))  -- ONE fused
                    DVE scalar_tensor_tensor op straight from PSUM
    - PE-transpose + free-dim reduce for the partition-axis max
    - d^2 = (B + |u|^2) - max;  sqrt + one Newton step
  entropy (bf16): ACT exp;  DVE scalar_tensor_tensor mult with
    accum_out => S = sum(exp(x)*x) in one fused op per tile
  normalization: TWO tiny AllReduce(max) collectives --
    u-stats early (hidden under distance chunks), d^2-stats at the end
    (sqrt/Newton for the per-row d overlaps the collective).

Self-contained: all shapes hardcoded; no sibling imports.
"""

import numpy as np

# ---- problem constants (hardcoded per contract) ----
N_U, N_L, NZ, C = 16384, 8192, 256, 1000
CORES = 8
MU = N_U // CORES          # 2048 rows of U / pred per core
P = 128                    # partitions
MT = MU // P               # 16 m-tiles per core
NCH = N_L // P             # 64 n-chunks of 128
MMN = 512                  # moving free dim per matmul (1 psum bank fp32)
EPS = 1e-18
L2C = 256.0                # centering constant for the l2 bias
FINF = 3.0e38

_CACHE = {}


def _is_stt(nb):
    """Chunks whose evacuation runs as a single fused DVE op from PSUM.
    The rest evacuate via ACT (relu transform) + DVE fp16 max.  Every
    4th chunk keeps both engines comfortably under the PE rate; the
    last chunks stay on ACT so acc finalizes right behind the last
    matmul (shorter pre-collective tail)."""
    return nb % 4 == 0 and 0 < nb < 57


def _build(lam: float, legalize: bool = True):
    import concourse.bass as bass
    import concourse.tile as tile
    from concourse import mybir

    f32 = mybir.dt.float32
    f16 = mybir.dt.float16
    bf16 = mybir.dt.bfloat16
    AX = mybir.AxisListType
    OP = mybir.AluOpType
    AF = mybir.ActivationFunctionType

    nc = bass.Bass(num_devices=CORES)

    # fp16 GEMM operands (fp32/fp32r matmuls run with a ~300ns self-load
    # per matmul; fp16 gets separate FWL weight loads).
    ut_h = nc.declare_dram_parameter("ut", [NZ, MU], f16, isOutput=False)    # +2*U.T
    lt_h = nc.declare_dram_parameter("lt", [NZ, N_L], f16, isOutput=False)   # L.T
    l2b_h = nc.declare_dram_parameter("l2b", [P, NCH], f32, isOutput=False)  # B-|l|^2
    u2_h = nc.declare_dram_parameter("u2c", [P, MT], f32, isOutput=False)    # B+|u|^2
    id_h = nc.declare_dram_parameter("idm", [P, P], f16, isOutput=False)     # identity
    id32_h = nc.declare_dram_parameter("idm32", [P, P], f32, isOutput=False)
    pr_h = nc.declare_dram_parameter("pred", [MU, C], f16, isOutput=False)
    out_h = nc.declare_dram_parameter("outv", [P, MT], f32, isOutput=True)

    ccu_in = nc.dram_tensor("ccu_in", [2], f32)
    ccu_out = nc.dram_tensor("ccu_out", [2], f32, addr_space="Shared")
    ccd_in = nc.dram_tensor("ccd_in", [2], f32)
    ccd_out = nc.dram_tensor("ccd_out", [2], f32, addr_space="Shared")

    from contextlib import ExitStack
    with tile.TileContext(nc) as tc, ExitStack() as stk:
        consts = stk.enter_context(tc.tile_pool(name="consts", bufs=1))
        preds = stk.enter_context(tc.tile_pool(name="preds", bufs=3))
        psums = stk.enter_context(tc.tile_pool(name="psums", bufs=4, space="PSUM"))
        evs = stk.enter_context(tc.tile_pool(name="evs", bufs=4))
        small = stk.enter_context(tc.tile_pool(name="small", bufs=1))

        # ---- resident SBUF tensors ----
        # few, large tiles: every dma_start costs ~0.6us of serialized
        # trigger time on the sync queue, so batch the streaming operands
        ut0 = consts.tile([P, MU], f16, tag="ut0")
        ut1 = consts.tile([P, MU], f16, tag="ut1")
        LTW = N_L // 4
        lt0c = [consts.tile([P, LTW], f16, tag=f"lt0_{q}", name=f"lt0_{q}") for q in range(4)]
        lt1c = [consts.tile([P, LTW], f16, tag=f"lt1_{q}", name=f"lt1_{q}") for q in range(4)]
        l2b = consts.tile([P, NCH], f32, tag="l2b")
        u2s = consts.tile([P, MT], f32, tag="u2s")
        idm = consts.tile([P, P], f16, tag="idm")
        idm32 = consts.tile([P, P], f32, tag="idm32")
        lt00 = consts.tile([P, P], f16, tag="lt00")     # chunk-0 weights, tiny
        lt10 = consts.tile([P, P], f16, tag="lt10")     #   -> earliest first MM
        # running max over n-chunks, split per half so the end-of-loop
        # transposes of half 0 can start while half 1 still evacuates
        acch = [consts.tile([P, MU // 2], f16, tag=f"acc{h}", name=f"acc{h}")
                for h in range(2)]

        S = small.tile([P, MT], f32, tag="S")           # sum(exp(x)*x) per row
        maxT = small.tile([P, MT], f32, tag="maxT")

        # first-needed operands first: each sem lane's FIFO head is what
        # the first n-chunk's matmuls actually wait on
        nc.sync.dma_start(out=lt00, in_=lt_h[0:P, 0:P])
        nc.sync.dma_start(out=lt10, in_=lt_h[P:NZ, 0:P])
        nc.sync.dma_start(out=ut0, in_=ut_h[0:P, :])
        nc.sync.dma_start(out=ut1, in_=ut_h[P:NZ, :])
        nc.sync.dma_start(out=lt0c[0], in_=lt_h[0:P, 0:LTW])
        nc.sync.dma_start(out=lt1c[0], in_=lt_h[P:NZ, 0:LTW])
        nc.sync.dma_start(out=l2b, in_=l2b_h[:])
        nc.sync.dma_start(out=u2s, in_=u2_h[:])
        nc.sync.dma_start(out=idm, in_=id_h[:])
        nc.sync.dma_start(out=idm32, in_=id32_h[:])

        # ---- distance: acc[p, m] = max over n-chunks of
        #      (B - l2[n]) + 2 u.l  with n = 128*chunk + p ----
        # psum tiles are half-chunks [P, MU/2] (2 banks) x4 bufs: the
        # 4-deep rotation lets PE run ~2 chunks ahead of the evacuation
        # engines, absorbing their jitter (exp tiles, STT ops).
        # Entropy tiles and the whole u-stats/AllReduce#1 chain are
        # emitted INSIDE the loop so they land mid-stream in each
        # engine's FIFO instead of after all 64 chunks.
        from concourse.tile import add_dep_helper
        MH = MU // 2
        chunk_mm = {}
        _pr = pr_h[:]
        t1 = small.tile([P, MT], f32, tag="t1")
        Gu = small.tile([P, 2], f32, tag="Gu")

        def emit_entropy(q):
            pt = preds.tile([P, 2 * C], f16, tag="pt")
            pd = nc.sync.dma_start(out=pt, in_=bass.AP(
                tensor=_pr.tensor, offset=_pr.offset + 2 * q * P * C,
                ap=[[C, P], [P * C, 2], [1, C]]))
            if q >= 1:
                gate = chunk_mm[4 * q + 1]
                add_dep_helper(pd.ins, gate.ins, sync=True,
                               reason="stage pred behind distance")
            et = preds.tile([P, 2 * C], f16, tag="et")
            nc.scalar.activation(et, pt, AF.Exp)
            xe = preds.tile([P, 2 * C], f16, tag="xe")
            for j in range(2):
                cs = slice(j * C, (j + 1) * C)
                nc.vector.scalar_tensor_tensor(
                    out=xe[:, cs], in0=et[:, cs], scalar=1.0, in1=pt[:, cs],
                    op0=OP.bypass, op1=OP.mult,
                    accum_out=S[:, 2 * q + j:2 * q + j + 1])

        def emit_ustats():
            # entropy stats -> AllReduce#1 -> hidden u-term of the output:
            # t1 = lam * (smax - S) / (smax - smin + eps)
            sneg = small.tile([P, MT], f32, tag="sneg")
            nc.vector.tensor_scalar_mul(sneg, S, -1.0)
            STu = small.tile([P, 2], f32, tag="STu")
            nc.vector.tensor_reduce(out=STu[:, 0:1], in_=S, axis=AX.X, op=OP.max)
            nc.vector.tensor_reduce(out=STu[:, 1:2], in_=sneg, axis=AX.X, op=OP.max)
            stups = psums.tile([2, P], f32, tag="ps")
            nc.tensor.transpose(stups, STu, idm32)
            STru = small.tile([2, 1], f32, tag="STru")
            nc.vector.tensor_reduce(out=STru, in_=stups, axis=AX.X, op=OP.max)
            nc.sync.dma_start(out=ccu_in[:], in_=STru)
            nc.gpsimd.collective_compute(
                "AllReduce", OP.max,
                replica_groups=[list(range(CORES))],
                ins=[ccu_in[:]], outs=[ccu_out[:]],
            )
            _cu = ccu_out[:]
            nc.sync.dma_start(out=Gu, in_=bass.AP(
                tensor=_cu.tensor, offset=_cu.offset,
                ap=[[0, P]] + [list(d) for d in _cu.ap]))
            # NOTE: the Gu-dependent DVE ops (su/ru/t1) are emitted after
            # the chunk loop -- placing them here would park a CC1-blocked
            # instruction in the strict DVE FIFO ahead of the remaining
            # chunk evacuations.

        # pull the (shared relu/exp) ACT table load to t=0: a dummy op on
        # an undefined scratch tile with no DMA dependencies
        warm = small.tile([P, 1], f32, tag="warm")
        nc.scalar.activation(warm, warm, AF.Relu)

        for nb in range(NCH):
            psh = [psums.tile([P, MH], f32, tag="ps", name=f"ps_{nb}_{h}")
                   for h in range(2)]
            first_mm = None
            for k in range(2):
                ltkc = lt0c if k == 0 else lt1c
                utk = ut0 if k == 0 else ut1
                if nb == 0:
                    w = lt00 if k == 0 else lt10
                else:
                    lo = (nb % 16) * P
                    w = ltkc[nb // 16][:, lo:lo + P]
                for h in range(2):
                    for s in range(2):
                        q = 2 * h + s
                        mm = nc.tensor.matmul(
                            psh[h][:, s * MMN:(s + 1) * MMN], w,
                            utk[:, q * MMN:(q + 1) * MMN],
                            start=(k == 0), stop=(k == 1))
                        if first_mm is None:
                            first_mm = mm
                            chunk_mm[nb] = mm
            # stream in the lt tile needed 8 n-chunks from now, gated on
            # this chunk's first matmul so early DMA bandwidth goes to the
            # operands needed first
            if nb in (8, 24, 40):
                q = nb // 16 + 1
                qs = slice(q * LTW, (q + 1) * LTW)
                d0 = nc.sync.dma_start(out=lt0c[q], in_=lt_h[0:P, qs])
                d1 = nc.sync.dma_start(out=lt1c[q], in_=lt_h[P:NZ, qs])
                add_dep_helper(d0.ins, first_mm.ins, sync=True,
                               reason="stage lt behind compute")
                add_dep_helper(d1.ins, first_mm.ins, sync=True,
                               reason="stage lt behind compute")
            for h in range(2):
                ah = acch[h][:]
                if nb == 0:
                    nc.scalar.activation(ah, psh[h], AF.Relu,
                                         bias=l2b[:, 0:1], scale=1.0)
                elif _is_stt(nb):
                    # fused transform + max-accumulate: ONE DVE op from PSUM
                    nc.vector.scalar_tensor_tensor(
                        out=ah, in0=psh[h], scalar=l2b[:, nb:nb + 1], in1=ah,
                        op0=OP.add, op1=OP.max)
                else:
                    ev = evs.tile([P, MH], f16, tag="ev")
                    nc.scalar.activation(ev, psh[h], AF.Relu,
                                         bias=l2b[:, nb:nb + 1], scale=1.0)
                    nc.vector.tensor_tensor(out=ah, in0=ah, in1=ev, op=OP.max)
            # entropy tile q rides behind chunk 4q+1 (DMA-gated + emitted
            # here so its ACT/DVE ops interleave at the right FIFO spot)
            if nb >= 1 and (nb - 1) % 4 == 0 and (nb - 1) // 4 < MT // 2:
                emit_entropy((nb - 1) // 4)
            # u-stats chain mid-stream: S is complete by chunk ~33
            if nb == 40:
                emit_ustats()

        # u-normalization factor + hidden u-term (CC1 done long ago):
        # t1 = lam * (smax - S) / (smax - smin + eps)
        su = small.tile([P, 1], f32, tag="su")
        nc.vector.tensor_add(su, Gu[:, 0:1], Gu[:, 1:2])       # smax - smin
        nc.vector.tensor_scalar_add(su, su, EPS)
        ru = small.tile([P, 1], f32, tag="ru")
        nc.vector.reciprocal(ru, su)
        nc.vector.tensor_scalar_mul(ru, ru, -lam)              # -lambda/span_u
        nc.vector.tensor_scalar(
            out=t1, in0=S, scalar1=Gu[:, 0:1], scalar2=ru,
            op0=OP.subtract, op1=OP.mult)

        # ---- partition-axis max via PE transpose ----
        for j in range(MT):
            tps = psums.tile([P, P], f16, tag="ps", name=f"tps{j}")
            ac = acch[j // (MT // 2)]
            jo = (j % (MT // 2)) * P
            nc.tensor.transpose(tps, ac[:, jo:jo + P], idm)
            nc.vector.tensor_reduce(
                out=maxT[:, j:j + 1], in_=tps, axis=AX.X, op=OP.max)

        d2 = small.tile([P, MT], f32, tag="d2")
        nc.vector.tensor_sub(d2, u2s, maxT)             # (B+u2) - max = min d^2
        nc.vector.tensor_scalar_max(d2, d2, 1e-12)

        # ---- d = sqrt(d^2) on DVE: bit-hack seed + 2 Newton steps ----
        # (keeps the sqrt table set off ACT entirely)
        yb = small.tile([P, MT], mybir.dt.uint32, tag="yb")
        nc.vector.tensor_scalar(
            out=yb, in0=d2.bitcast(mybir.dt.uint32),
            scalar1=1, scalar2=None, op0=OP.arith_shift_right)
        nc.vector.tensor_scalar_add(yb, yb, 0x1fbd1df5)
        dv = yb.bitcast(f32)                            # ~sqrt, +-3.5%
        rc = small.tile([P, MT], f32, tag="rc")
        xy = small.tile([P, MT], f32, tag="xy")
        for _ in range(2):
            nc.vector.reciprocal(rc, dv)
            nc.vector.tensor_mul(xy, rc, d2)            # d2 / y
            nc.vector.tensor_add(dv, dv, xy)
            nc.vector.tensor_scalar_mul(dv, dv, 0.5)    # y = (y + d2/y)/2

        # ---- d stats -> LATE AllReduce (2 floats) ----
        dneg = small.tile([P, MT], f32, tag="dneg")
        nc.vector.tensor_scalar_mul(dneg, dv, -1.0)
        STd = small.tile([P, 2], f32, tag="STd")
        nc.vector.tensor_reduce(out=STd[:, 0:1], in_=dv, axis=AX.X, op=OP.max)
        nc.vector.tensor_reduce(out=STd[:, 1:2], in_=dneg, axis=AX.X, op=OP.max)
        stdps = psums.tile([2, P], f32, tag="ps")
        nc.tensor.transpose(stdps, STd, idm32)
        STrd = small.tile([2, 1], f32, tag="STrd")
        nc.vector.tensor_reduce(out=STrd, in_=stdps, axis=AX.X, op=OP.max)
        nc.sync.dma_start(out=ccd_in[:], in_=STrd)
        nc.gpsimd.collective_compute(
            "AllReduce", OP.max,
            replica_groups=[list(range(CORES))],
            ins=[ccd_in[:]], outs=[ccd_out[:]],
        )
        Gd = small.tile([P, 2], f32, tag="Gd")
        _cd = ccd_out[:]
        nc.sync.dma_start(out=Gd, in_=bass.AP(
            tensor=_cd.tensor, offset=_cd.offset,
            ap=[[0, P]] + [list(d) for d in _cd.ap]))

        # span + reciprocal, then out = t1 + (d - dmin)/span_d
        sd = small.tile([P, 1], f32, tag="sd")
        nc.vector.tensor_add(sd, Gd[:, 0:1], Gd[:, 1:2])     # dmax - dmin
        nc.vector.tensor_scalar_add(sd, sd, EPS)
        rd = small.tile([P, 1], f32, tag="rd")
        nc.vector.reciprocal(rd, sd)
        t2 = small.tile([P, MT], f32, tag="t2")
        nc.vector.tensor_scalar(
            out=t2, in0=dv, scalar1=Gd[:, 1:2], scalar2=rd,
            op0=OP.add, op1=OP.mult)
        ov = small.tile([P, MT], f32, tag="ov")
        nc.vector.tensor_add(ov, t1, t2)
        nc.sync.dma_start(out=out_h[:], in_=ov)

    _dedupe_ldweights(nc)
    if legalize:
        _split_multi_waits(nc, mybir)
    return nc


def _dedupe_ldweights(nc):
    """Consecutive PE matmuls over the same stationary tile each get their
    own InstLdweights from tile_legalize; the array state is unchanged, so
    drop the repeats (moving their sync info to the next PE instruction)."""
    import concourse.mybir as mybir
    PE = mybir.EngineType.PE
    for func in nc.m.functions:
        for block in func.blocks:
            out = []
            changed = False
            last_key = None
            pending = []            # sync entries from dropped LDWs
            for inst in block.instructions:
                if inst.engine != PE:
                    out.append(inst)
                    continue
                if isinstance(inst, mybir.InstLdweights):
                    key = str(inst.ins)
                    if key == last_key:
                        si = inst.sync_info
                        if si is not None:
                            pending.extend(list(si.on_wait or []))
                            pending.extend(
                                ("upd", u) for u in (si.on_update or []))
                        changed = True
                        continue
                    last_key = key
                if pending:
                    si = inst.sync_info
                    waits = list(si.on_wait or []) if si is not None else []
                    upds = list(si.on_update or []) if si is not None else []
                    for p in pending:
                        if isinstance(p, tuple):
                            upds.append(p[1])
                        else:
                            waits.append(p)
                    inst.sync_info = mybir.SyncInfo(on_wait=waits, on_update=upds)
                    pending = []
                out.append(inst)
            if changed:
                block.instructions = out


def _split_multi_waits(nc, mybir):
    """This walrus build accepts at most ONE sync-wait command per
    instruction; Tile freely attaches several.  Hoist all but the last
    wait onto dedicated same-engine NoOps inserted just before."""
    n = 0
    for func in nc.m.functions:
        for block in func.blocks:
            out = []
            changed = False
            for inst in block.instructions:
                si = inst.sync_info
                waits = list(si.on_wait) if si is not None and si.on_wait else []
                if len(waits) > 1:
                    for w in waits[:-1]:
                        nop = mybir.InstNoOp(name=f"WSPLIT-{n}", ins=[], outs=[])
                        n += 1
                        nop.engine = inst.engine
                        nop.sync_info = mybir.SyncInfo(on_wait=[w], on_update=[])
                        out.append(nop)
                    inst.sync_info = mybir.SyncInfo(
                        on_wait=[waits[-1]],
                        on_update=list(si.on_update or []))
                    changed = True
                out.append(inst)
            if changed:
                block.instructions = out


def _prep_inputs(pred, U_z, L_z):
    f = np.float32
    h = np.float16
    pred = np.asarray(pred, dtype=f)
    U = np.asarray(U_z, dtype=f)
    L = np.asarray(L_z, dtype=f)
    lt = np.ascontiguousarray(L.T.astype(h))             # [NZ, N_L] fp16
    l2 = (L * L).sum(axis=1).astype(f)                   # [N_L]
    l2bias = np.ascontiguousarray(
        (np.float32(L2C) - l2).reshape(NCH, P).T)        # [P, NCH]
    idm = np.eye(P, dtype=h)
    in_maps = []
    for c in range(CORES):
        r = slice(c * MU, (c + 1) * MU)
        Uc = U[r]
        in_maps.append({
            "ut": np.ascontiguousarray((2.0 * Uc).T.astype(h)),   # [NZ, MU]
            "lt": lt,
            "l2b": l2bias,
            "u2c": np.ascontiguousarray(
                ((Uc * Uc).sum(axis=1).astype(f) + np.float32(L2C))
                .reshape(MT, P).T),
            "idm": idm,
            "idm32": np.eye(P, dtype=f),
            "pred": np.ascontiguousarray(pred[r].astype(h)),
        })
    return in_maps


def _run(pred, U_z, L_z, lambda_, trace=False):
    from concourse import bass_utils
    lam = float(lambda_)
    key = lam
    if key not in _CACHE:
        _CACHE[key] = _build(lam)
    nc = _CACHE[key]
    in_maps = _prep_inputs(pred, U_z, L_z)
    res = bass_utils.run_bass_kernel_spmd(
        nc, in_maps, list(range(CORES)), trace=trace)
    out = np.empty(N_U, dtype=np.float32)
    for c in range(CORES):
        ov = res.results[c]["outv"]                      # [P, MT]
        out[c * MU:(c + 1) * MU] = ov.T.reshape(MU)
    return out, res


def kernel(pred, U_z, L_z, lambda_):
    out, _ = _run(pred, U_z, L_z, lambda_)
    return out
